# revision 16
# baseline (speedup 1.0000x reference)
"""Trainium2 Bass kernel for nn_Contrast_loss (B=8192, D=256, 100 classes).

Math: with mask = -same + l*(1-same) + I and same_ii = 1,
    loss = l*||s||^2 - (1+l)*sum_c ||g_c||^2 + sum_i ||f_i||^2
where s = sum_i f_i and g_c = sum_{i: label_i = c} f_i.

Every term decomposes over feature columns, so feat is sharded
column-wise across the 8 cores (32 columns each); the host sums the 8
partial scalars. No cross-core collective.

Layout: the loss is invariant to row permutations, so the host sorts
classes by count and packs them into 25 groups of 4; group q, lane m
holds sorted class 4q+m at partition p = 32*m + col. Groups are padded
to a per-bucket slot count (NB=4 buckets split by group max count),
which cuts the padded free dim from 25*max_count (2800 at 112 slots)
to ~sum of bucket maxes (2160 for the seed-0 label distribution).

Device per core: one bf16 tile [128, TOTAL]; the four buckets arrive
as four DMAs alternating the sync/scalar HWDGE rings so compute starts
on bucket 1 while the rest are in flight (each piece's completion also
absorbs the chronically slow SDMA engine 15 once, not per-kernel).
  - DVE: per-bucket tensor_reduce over the slot axis -> class sums g,
    then sg = reduce(g) and -1.5*sum g^2 via affine_mul_reduce.
  - ACT: Square with accum_out in 2 pieces (buckets 0-1 | 2-3)
    -> sum f^2 partials; the table load overlaps the input DMA.
  - out [128, 4] fp32 in one sync-ring DMA:
    [-(1+l)*sum g^2, sg, acc_piece0, acc_piece1].
Host: loss = sum_cores [col0 + col2 + col3] + l * sum_j (fold col1)^2.
"""

import numpy as np
import ml_dtypes

import concourse.bacc as bacc
import concourse.mybir as mybir
import concourse.tile as tile
from concourse import bass_utils

B = 8192
D = 256
N_CORES = 8
DPC = D // N_CORES          # 32 feature columns per core
P = 128                     # partitions
NCLS = 100
MG = 4                      # classes per partition-lane group
Q = NCLS // MG              # 25 class groups
LAMDA = 0.5
NB = 4                      # padding buckets

FP32 = mybir.dt.float32
BF16 = mybir.dt.bfloat16

_CACHED = {}


def _build_nc(spec):
    """spec: tuple of (nq, s) per bucket."""
    total = sum(nq * s for nq, s in spec)
    nc = bacc.Bacc("TRN2", target_bir_lowering=False, debug=False,
                   num_devices=N_CORES)

    feat_d = nc.dram_tensor("feat", [P, total], BF16, kind="ExternalInput")
    NOUT = 4
    out_d = nc.dram_tensor("out", [P, NOUT], FP32, kind="ExternalOutput")

    with tile.TileContext(nc) as tc:
        with (
            tc.tile_pool(name="big", bufs=1) as big,
            tc.tile_pool(name="small", bufs=1) as small,
        ):
            fpad = big.tile([P, total], BF16)
            sq_t = big.tile([P, total], BF16)     # Act square scratch (dead)
            g_t = small.tile([P, Q], FP32)
            sqg_t = small.tile([P, Q], FP32)      # affine_mul scratch (dead)
            outt = small.tile([P, NOUT], FP32)

            src = feat_d.rearrange("p x -> p x")
            offs = []
            off = 0
            for b, (nq, s) in enumerate(spec):
                eng = nc.sync if b % 2 == 0 else nc.scalar
                eng.dma_start(fpad[:, off:off + nq * s],
                              src[:, off:off + nq * s])
                offs.append(off)
                off += nq * s
            # DVE: class sums per bucket
            qoff = 0
            for (nq, s), off in zip(spec, offs):
                nc.vector.tensor_reduce(
                    g_t[:, qoff:qoff + nq],
                    fpad[:, off:off + nq * s].rearrange("p (q s) -> p q s",
                                                        q=nq),
                    mybir.AxisListType.X, mybir.AluOpType.add)
                qoff += nq

            # ACT: squares with accumulate, 2 pieces (buckets 0-1 | 2-3)
            # balanced so piece B holds minimal work behind the last sem
            cut = spec[0][0] * spec[0][1] + spec[1][0] * spec[1][1]
            nc.scalar.activation(sq_t[:, 0:cut], fpad[:, 0:cut],
                                 mybir.ActivationFunctionType.Square,
                                 accum_out=outt[:, 2:3])
            nc.scalar.activation(sq_t[:, cut:total], fpad[:, cut:total],
                                 mybir.ActivationFunctionType.Square,
                                 accum_out=outt[:, 3:4])

            # epilogue: sg and -(1+l)*sum g^2 (both DVE)
            nc.vector.tensor_reduce(outt[:, 1:2], g_t[:],
                                    mybir.AxisListType.X, mybir.AluOpType.add)
            nc.vector.affine_mul_reduce(sqg_t[:], outt[:, 0:1], g_t[:], g_t[:],
                                        -(1.0 + LAMDA), 0.0)

            nc.sync.dma_start(out_d[:], outt[:])

    nc.compile()
    return nc


def _get_nc(spec):
    if spec not in _CACHED:
        _CACHED[spec] = _build_nc(spec)
    return _CACHED[spec]


def _bucket_spec(gmax):
    """Split the Q sorted group-max counts into NB contiguous buckets
    minimizing total padded columns. Returns tuple of (nq, slots)."""
    import itertools
    best = None
    for cuts in itertools.combinations(range(1, Q), NB - 1):
        bounds = [0] + list(cuts) + [Q]
        tot = 0
        spec = []
        for a, b in zip(bounds, bounds[1:]):
            s = int(gmax[a:b].max())
            s = max(2, (s + 1) // 2 * 2)
            tot += (b - a) * s
            spec.append((b - a, s))
        if best is None or tot < best[0]:
            best = (tot, tuple(spec))
    return best[1]


def _prep(feat, label):
    feat = np.asarray(feat, dtype=np.float32)
    label = np.asarray(label).astype(np.int64).ravel()
    cnt = np.bincount(label, minlength=NCLS)

    # sorted-by-count class order; class at order position i -> group
    # q = i//4, lane m = i%4
    cls_order = np.argsort(cnt, kind="stable")
    pos_of_cls = np.empty(NCLS, dtype=np.int64)
    pos_of_cls[cls_order] = np.arange(NCLS)
    gmax = np.sort(cnt).reshape(Q, MG).max(1)
    spec = _bucket_spec(gmax)

    # per-row placement
    order = np.argsort(label, kind="stable")
    lab_s = label[order]
    start = np.zeros(NCLS, dtype=np.int64)
    start[1:] = np.cumsum(cnt)[:-1]
    slot = np.arange(B) - start[lab_s]

    i_of_row = pos_of_cls[lab_s]           # order position of row's class
    q_of_row = i_of_row // MG
    m_of_row = i_of_row % MG

    # group q -> (bucket, colbase)
    colbase = np.zeros(Q, dtype=np.int64)
    off = 0
    qa = 0
    for nq, s in spec:
        for qq in range(nq):
            colbase[qa + qq] = off + qq * s
        off += nq * s
        qa += nq
    total = off

    col_of_row = colbase[q_of_row] + slot
    part_base = m_of_row * DPC             # + feature lane j

    dev = np.zeros((P, total, N_CORES), dtype=np.float32)
    fs = feat[order].reshape(B, N_CORES, DPC)   # [row, core, j]
    # dev[part_base+j, col, core] = fs[row, core, j]
    for j in range(DPC):
        dev[part_base + j, col_of_row, :] = fs[:, :, j]
    dev = np.ascontiguousarray(
        dev.transpose(2, 0, 1).astype(ml_dtypes.bfloat16))
    return dev, spec

def kernel(feat, label, _trace=False):
    dev, spec = _prep(feat, label)
    nc = _get_nc(spec)
    in_maps = [{"feat": dev[m]} for m in range(N_CORES)]
    res = bass_utils.run_bass_kernel_spmd(
        nc, in_maps, core_ids=list(range(N_CORES)), trace=_trace)
    total = np.float64(0.0)
    for r in res.results:
        o = np.asarray(r["out"], dtype=np.float64)
        total += o[:, 0].sum() + o[:, 2].sum() + o[:, 3].sum()
        sj = o[:, 1].reshape(MG, DPC).sum(axis=0)
        total += LAMDA * np.square(sj).sum()
    out = np.float32(total)
    if _trace:
        return out, res
    return out


# revision 17
# speedup vs baseline: 1.1003x; 1.1003x over previous
"""Trainium2 Bass kernel for nn_Contrast_loss (B=8192, D=256, 100 classes).

Math: with mask = -same + l*(1-same) + I and same_ii = 1,
    loss = l*||s||^2 - (1+l)*sum_c ||g_c||^2 + sum_i ||f_i||^2
where s = sum_i f_i and g_c = sum_{i: label_i = c} f_i.

Every term decomposes over feature columns, so feat is sharded
column-wise across the 8 cores (32 columns each); the host sums the 8
partial scalars. No cross-core collective.

Layout: the loss is invariant to row permutations, so the host sorts
classes by count and packs them into 25 groups of 4; group q, lane m
holds sorted class 4q+m at partition p = 32*m + col. Groups are padded
to a per-bucket slot count (NB=4 buckets split by group max count),
which cuts the padded free dim from 25*max_count (2800 at 112 slots)
to ~sum of bucket maxes (2160 for the seed-0 label distribution).

Device per core: one bf16 tile [128, TOTAL]; the four buckets arrive
as four DMAs alternating the sync/scalar HWDGE rings so compute starts
on bucket 1 while the rest are in flight (each piece's completion also
absorbs the chronically slow SDMA engine 15 once, not per-kernel).
  - DVE: per-bucket tensor_reduce over the slot axis -> class sums g,
    then sg = reduce(g) and -1.5*sum g^2 via affine_mul_reduce.
  - ACT: Square with accum_out in 2 pieces (buckets 0-1 | 2-3)
    -> sum f^2 partials; the table load overlaps the input DMA.
  - out [128, 4] fp32 in one sync-ring DMA:
    [-(1+l)*sum g^2, sg, acc_piece0, acc_piece1].
Host: loss = sum_cores [col0 + col2 + col3] + l * sum_j (fold col1)^2.
"""

import numpy as np
import ml_dtypes

import concourse.bacc as bacc
import concourse.mybir as mybir
import concourse.tile as tile
from concourse import bass_utils

B = 8192
D = 256
N_CORES = 8
DPC = D // N_CORES          # 32 feature columns per core
P = 128                     # partitions
NCLS = 100
MG = 4                      # classes per partition-lane group
Q = NCLS // MG              # 25 class groups
LAMDA = 0.5
NB = 4                      # padding buckets

FP32 = mybir.dt.float32
BF16 = mybir.dt.bfloat16

_CACHED = {}


def _build_nc(spec):
    """spec: tuple of (nq, s) per bucket."""
    total = sum(nq * s for nq, s in spec)
    nc = bacc.Bacc("TRN2", target_bir_lowering=False, debug=False,
                   num_devices=N_CORES)

    feat_d = nc.dram_tensor("feat", [P, total], BF16, kind="ExternalInput")
    NOUT = 4
    out_d = nc.dram_tensor("out", [P, NOUT], FP32, kind="ExternalOutput")

    with tile.TileContext(nc) as tc:
        with (
            tc.tile_pool(name="big", bufs=1) as big,
            tc.tile_pool(name="small", bufs=1) as small,
        ):
            fpad = big.tile([P, total], BF16)
            sq_t = big.tile([P, total], BF16)     # Act square scratch (dead)
            g_t = small.tile([P, Q], FP32)
            sqg_t = small.tile([P, Q], FP32)      # affine_mul scratch (dead)
            outt = small.tile([P, NOUT], FP32)

            src = feat_d.rearrange("p x -> p x")
            offs = []
            off = 0
            for b, (nq, s) in enumerate(spec):
                eng = nc.sync if b % 2 == 0 else nc.scalar
                eng.dma_start(fpad[:, off:off + nq * s],
                              src[:, off:off + nq * s])
                offs.append(off)
                off += nq * s
            # DVE: class sums per bucket; b4 (small, lands early on the
            # scalar ring) is reduced before b3 (big, lands last)
            qoffs = []
            qoff = 0
            for nq, s in spec:
                qoffs.append(qoff)
                qoff += nq
            for b in (0, 1, 3, 2):
                nq, s = spec[b]
                nc.vector.tensor_reduce(
                    g_t[:, qoffs[b]:qoffs[b] + nq],
                    fpad[:, offs[b]:offs[b] + nq * s].rearrange(
                        "p (q s) -> p q s", q=nq),
                    mybir.AxisListType.X, mybir.AluOpType.add)

            # ACT: squares with accumulate, 2 pieces (buckets 0-1 | 2-3)
            # balanced so piece B holds minimal work behind the last sem
            cut = spec[0][0] * spec[0][1] + spec[1][0] * spec[1][1]
            nc.scalar.activation(sq_t[:, 0:cut], fpad[:, 0:cut],
                                 mybir.ActivationFunctionType.Square,
                                 accum_out=outt[:, 2:3])
            nc.scalar.activation(sq_t[:, cut:total], fpad[:, cut:total],
                                 mybir.ActivationFunctionType.Square,
                                 accum_out=outt[:, 3:4])

            # epilogue: sg and -(1+l)*sum g^2 (both DVE)
            nc.vector.tensor_reduce(outt[:, 1:2], g_t[:],
                                    mybir.AxisListType.X, mybir.AluOpType.add)
            nc.vector.affine_mul_reduce(sqg_t[:], outt[:, 0:1], g_t[:], g_t[:],
                                        -(1.0 + LAMDA), 0.0)

            nc.sync.dma_start(out_d[:], outt[:])

    nc.compile()
    return nc


def _get_nc(spec):
    if spec not in _CACHED:
        _CACHED[spec] = _build_nc(spec)
    return _CACHED[spec]


def _bucket_spec(gmax):
    """Split the Q sorted group-max counts into NB contiguous buckets.
    Fixed pipeline-shaped cuts: small b1 (starts DVE early), small b2
    (starts ACT early), big b3 (lands last, on the sync ring), small b4
    (lands early on the scalar ring; DVE reduces it before b3 so only
    the b3 reduce plus the epilogue sit behind the final DMA sem)."""
    bounds = [0, 4, 10, 21, Q]
    spec = []
    for a, b in zip(bounds, bounds[1:]):
        s = int(gmax[a:b].max())
        s = max(2, (s + 1) // 2 * 2)
        spec.append((b - a, s))
    return tuple(spec)


def _prep(feat, label):
    feat = np.asarray(feat, dtype=np.float32)
    label = np.asarray(label).astype(np.int64).ravel()
    cnt = np.bincount(label, minlength=NCLS)

    # sorted-by-count class order; class at order position i -> group
    # q = i//4, lane m = i%4
    cls_order = np.argsort(cnt, kind="stable")
    pos_of_cls = np.empty(NCLS, dtype=np.int64)
    pos_of_cls[cls_order] = np.arange(NCLS)
    gmax = np.sort(cnt).reshape(Q, MG).max(1)
    spec = _bucket_spec(gmax)

    # per-row placement
    order = np.argsort(label, kind="stable")
    lab_s = label[order]
    start = np.zeros(NCLS, dtype=np.int64)
    start[1:] = np.cumsum(cnt)[:-1]
    slot = np.arange(B) - start[lab_s]

    i_of_row = pos_of_cls[lab_s]           # order position of row's class
    q_of_row = i_of_row // MG
    m_of_row = i_of_row % MG

    # group q -> (bucket, colbase)
    colbase = np.zeros(Q, dtype=np.int64)
    off = 0
    qa = 0
    for nq, s in spec:
        for qq in range(nq):
            colbase[qa + qq] = off + qq * s
        off += nq * s
        qa += nq
    total = off

    col_of_row = colbase[q_of_row] + slot
    part_base = m_of_row * DPC             # + feature lane j

    dev = np.zeros((P, total, N_CORES), dtype=np.float32)
    fs = feat[order].reshape(B, N_CORES, DPC)   # [row, core, j]
    # dev[part_base+j, col, core] = fs[row, core, j]
    for j in range(DPC):
        dev[part_base + j, col_of_row, :] = fs[:, :, j]
    dev = np.ascontiguousarray(
        dev.transpose(2, 0, 1).astype(ml_dtypes.bfloat16))
    return dev, spec

def kernel(feat, label, _trace=False):
    dev, spec = _prep(feat, label)
    nc = _get_nc(spec)
    in_maps = [{"feat": dev[m]} for m in range(N_CORES)]
    res = bass_utils.run_bass_kernel_spmd(
        nc, in_maps, core_ids=list(range(N_CORES)), trace=_trace)
    total = np.float64(0.0)
    for r in res.results:
        o = np.asarray(r["out"], dtype=np.float64)
        total += o[:, 0].sum() + o[:, 2].sum() + o[:, 3].sum()
        sj = o[:, 1].reshape(MG, DPC).sum(axis=0)
        total += LAMDA * np.square(sj).sum()
    out = np.float32(total)
    if _trace:
        return out, res
    return out


# revision 18
# speedup vs baseline: 1.1293x; 1.0264x over previous
"""Trainium2 Bass kernel for nn_Contrast_loss (B=8192, D=256, 100 classes).

Math: with mask = -same + l*(1-same) + I and same_ii = 1,
    loss = l*||s||^2 - (1+l)*sum_c ||g_c||^2 + sum_i ||f_i||^2
where s = sum_i f_i and g_c = sum_{i: label_i = c} f_i.

Every term decomposes over feature columns, so feat is sharded
column-wise across the 8 cores (32 columns each); the host sums the 8
partial scalars. No cross-core collective.

Layout: the loss is invariant to row permutations, so the host sorts
classes by count and packs them into 25 groups of 4; group q, lane m
holds sorted class 4q+m at partition p = 32*m + col. Groups are padded
to a per-bucket slot count (NB=4 buckets split by group max count),
which cuts the padded free dim from 25*max_count (2800 at 112 slots)
to ~sum of bucket maxes (2160 for the seed-0 label distribution).

Device per core: one bf16 tile [128, TOTAL]; the four buckets arrive
as four DMAs alternating the sync/scalar HWDGE rings so compute starts
on bucket 1 while the rest are in flight (each piece's completion also
absorbs the chronically slow SDMA engine 15 once, not per-kernel).
  - DVE: per-bucket tensor_reduce over the slot axis -> class sums g,
    then sg = reduce(g) and -1.5*sum g^2 via affine_mul_reduce.
  - ACT: Square with accum_out in 2 pieces (buckets 0-1 | 2-3)
    -> sum f^2 partials; the table load overlaps the input DMA.
  - out [128, 4] fp32 in one sync-ring DMA:
    [-(1+l)*sum g^2, sg, acc_piece0, acc_piece1].
Host: loss = sum_cores [col0 + col2 + col3] + l * sum_j (fold col1)^2.
"""

import numpy as np
import ml_dtypes

import concourse.bacc as bacc
import concourse.mybir as mybir
import concourse.tile as tile
from concourse import bass_utils

B = 8192
D = 256
N_CORES = 8
DPC = D // N_CORES          # 32 feature columns per core
P = 128                     # partitions
NCLS = 100
MG = 4                      # classes per partition-lane group
Q = NCLS // MG              # 25 class groups
LAMDA = 0.5
NB = 4                      # padding buckets

FP32 = mybir.dt.float32
BF16 = mybir.dt.bfloat16

_CACHED = {}


def _build_nc(spec):
    """spec: tuple of (nq, s) per bucket."""
    total = sum(nq * s for nq, s in spec)
    nc = bacc.Bacc("TRN2", target_bir_lowering=False, debug=False,
                   num_devices=N_CORES)

    feat_d = nc.dram_tensor("feat", [P, total], BF16, kind="ExternalInput")
    NOUT = 4
    out_d = nc.dram_tensor("out", [P, NOUT], FP32, kind="ExternalOutput")

    with tile.TileContext(nc) as tc:
        with (
            tc.tile_pool(name="big", bufs=1) as big,
            tc.tile_pool(name="small", bufs=1) as small,
        ):
            fpad = big.tile([P, total], BF16)
            sq_t = big.tile([P, total], BF16)     # Act square scratch (dead)
            g_t = small.tile([P, Q], FP32)
            sqg_t = small.tile([P, Q], FP32)      # affine_mul scratch (dead)
            outt = small.tile([P, NOUT], FP32)

            src = feat_d.rearrange("p x -> p x")
            offs = []
            off = 0
            for b, (nq, s) in enumerate(spec):
                eng = nc.sync if b % 2 == 0 else nc.scalar
                eng.dma_start(fpad[:, off:off + nq * s],
                              src[:, off:off + nq * s])
                offs.append(off)
                off += nq * s
            # DVE: class sums per bucket
            qoff = 0
            for (nq, s), off in zip(spec, offs):
                nc.vector.tensor_reduce(
                    g_t[:, qoff:qoff + nq],
                    fpad[:, off:off + nq * s].rearrange("p (q s) -> p q s",
                                                        q=nq),
                    mybir.AxisListType.X, mybir.AluOpType.add)
                qoff += nq

            # ACT: squares with accumulate, 2 pieces (buckets 0-1 | 2-3)
            # balanced so piece B holds minimal work behind the last sem
            cut = spec[0][0] * spec[0][1] + spec[1][0] * spec[1][1]
            nc.scalar.activation(sq_t[:, 0:cut], fpad[:, 0:cut],
                                 mybir.ActivationFunctionType.Square,
                                 accum_out=outt[:, 2:3])
            nc.scalar.activation(sq_t[:, cut:total], fpad[:, cut:total],
                                 mybir.ActivationFunctionType.Square,
                                 accum_out=outt[:, 3:4])

            # epilogue: sg and -(1+l)*sum g^2 (both DVE)
            nc.vector.tensor_reduce(outt[:, 1:2], g_t[:],
                                    mybir.AxisListType.X, mybir.AluOpType.add)
            nc.vector.affine_mul_reduce(sqg_t[:], outt[:, 0:1], g_t[:], g_t[:],
                                        -(1.0 + LAMDA), 0.0)

            nc.sync.dma_start(out_d[:], outt[:])

    nc.compile()
    return nc


def _get_nc(spec):
    if spec not in _CACHED:
        _CACHED[spec] = _build_nc(spec)
    return _CACHED[spec]


def _bucket_spec(gmax):
    """Split the Q sorted group-max counts into NB contiguous buckets
    minimizing total padded columns. Returns tuple of (nq, slots)."""
    import itertools
    best = None
    for cuts in itertools.combinations(range(1, Q), NB - 1):
        bounds = [0] + list(cuts) + [Q]
        tot = 0
        spec = []
        for a, b in zip(bounds, bounds[1:]):
            s = int(gmax[a:b].max())
            s = max(2, (s + 1) // 2 * 2)
            tot += (b - a) * s
            spec.append((b - a, s))
        if best is None or tot < best[0]:
            best = (tot, tuple(spec))
    return best[1]


def _prep(feat, label):
    feat = np.asarray(feat, dtype=np.float32)
    label = np.asarray(label).astype(np.int64).ravel()
    cnt = np.bincount(label, minlength=NCLS)

    # sorted-by-count class order; class at order position i -> group
    # q = i//4, lane m = i%4
    cls_order = np.argsort(cnt, kind="stable")
    pos_of_cls = np.empty(NCLS, dtype=np.int64)
    pos_of_cls[cls_order] = np.arange(NCLS)
    gmax = np.sort(cnt).reshape(Q, MG).max(1)
    spec = _bucket_spec(gmax)

    # per-row placement
    order = np.argsort(label, kind="stable")
    lab_s = label[order]
    start = np.zeros(NCLS, dtype=np.int64)
    start[1:] = np.cumsum(cnt)[:-1]
    slot = np.arange(B) - start[lab_s]

    i_of_row = pos_of_cls[lab_s]           # order position of row's class
    q_of_row = i_of_row // MG
    m_of_row = i_of_row % MG

    # group q -> (bucket, colbase)
    colbase = np.zeros(Q, dtype=np.int64)
    off = 0
    qa = 0
    for nq, s in spec:
        for qq in range(nq):
            colbase[qa + qq] = off + qq * s
        off += nq * s
        qa += nq
    total = off

    col_of_row = colbase[q_of_row] + slot
    part_base = m_of_row * DPC             # + feature lane j

    dev = np.zeros((P, total, N_CORES), dtype=np.float32)
    fs = feat[order].reshape(B, N_CORES, DPC)   # [row, core, j]
    # dev[part_base+j, col, core] = fs[row, core, j]
    for j in range(DPC):
        dev[part_base + j, col_of_row, :] = fs[:, :, j]
    dev = np.ascontiguousarray(
        dev.transpose(2, 0, 1).astype(ml_dtypes.bfloat16))
    return dev, spec

def kernel(feat, label, _trace=False):
    dev, spec = _prep(feat, label)
    nc = _get_nc(spec)
    in_maps = [{"feat": dev[m]} for m in range(N_CORES)]
    res = bass_utils.run_bass_kernel_spmd(
        nc, in_maps, core_ids=list(range(N_CORES)), trace=_trace)
    total = np.float64(0.0)
    for r in res.results:
        o = np.asarray(r["out"], dtype=np.float64)
        total += o[:, 0].sum() + o[:, 2].sum() + o[:, 3].sum()
        sj = o[:, 1].reshape(MG, DPC).sum(axis=0)
        total += LAMDA * np.square(sj).sum()
    out = np.float32(total)
    if _trace:
        return out, res
    return out
